# revision 1
# baseline (speedup 1.0000x reference)
"""Trainium2 Bass kernel for CyclicShiftConv (Hilbert-rotation SE attention).

out[b,c,l] = sum_r softmax_r(MLP(mean_l x[b,c,rot_idx[r,l]]))[b,c,r] * x[b,c,rot_idx[r,l]]

Strategy (8 cores, data-parallel over batch; 2 samples = 512 (b,c) rows/core):
  - The gather indices rot_idx[r, l] are SHARED across all (b,c) rows, so the
    gather is done row-wise in transposed layout: X^T[l, bc] rows are 2 KiB
    contiguous chunks, gathered with the GPSIMD dma_gather engine.
  - mean over l of the gathered tensor == x @ (bincount(rot_idx[r])/L), so the
    SE-MLP input is a tiny matmul against a host-precomputed count vector; no
    gather needed before the MLP.
  - Gathered rows are returned to natural [bc, l] layout with PE transpose-mode
    matmuls (one PSUM tile per rotation); the softmax weights are applied and
    the 4 rotations summed by a scalar_tensor_tensor chain reading PSUM with
    per-partition weight scalars.
"""

import sys

for _p in ("/opt/trn_rl_repo", "/opt/pypackages"):
    if _p not in sys.path:
        sys.path.append(_p)

import numpy as np

B, C, L, R, RED = 16, 256, 4096, 4, 16
NCORES = 8
BPC = B // NCORES          # samples per core
BC = BPC * C               # 512 rows per core
NT = L // 128              # 32 l-tiles
M_CHUNK = 256              # l-positions per gather chunk
NM = L // M_CHUNK          # 16 chunks
GIDX = R * M_CHUNK         # 1024 gather indices per chunk (4 rotations concat)

# merged f32 constant block: [128, CST_W]
CST_ID = 0                 # identity [128, 128]
CST_CNT = 128              # cnt      [128, 128]
CST_W1 = 256               # w1       [128, 32]
CST_B2 = 288               # b2       [128, 2]
CST_W2 = 290               # w2       [16, 256] (partitions 0:16)
CST_B1 = 546               # b1       [16, 1]
CST_SEL = 547              # row-selector [4, 512]: sel[r, r*128:(r+1)*128]=1
CST_W = 1059

_NC_CACHE = {}


def _build_nc(loop_n=1):
    import concourse.bass as bass
    import concourse.mybir as mybir
    from concourse import bacc
    from concourse.tile import TileContext
    from contextlib import ExitStack

    f32 = mybir.dt.float32
    i16 = mybir.dt.int16
    AF = mybir.ActivationFunctionType
    AX = mybir.AxisListType
    ALU = mybir.AluOpType

    nc = bacc.Bacc(
        "TRN2",
        target_bir_lowering=False,
        debug=False,
        enable_asserts=False,
        num_devices=NCORES,
    )

    x_in = nc.dram_tensor("x", [BC, L], f32, kind="ExternalInput").ap()
    cst_in = nc.dram_tensor("cst", [128, CST_W], f32, kind="ExternalInput").ap()
    idx_in = nc.dram_tensor("idx", [128, NM * (GIDX // 16)], i16, kind="ExternalInput").ap()
    out = nc.dram_tensor("out", [BC, L], f32, kind="ExternalOutput").ap()

    with TileContext(nc) as tc, ExitStack() as ctx:
        if loop_n > 1:
            ctx.enter_context(tc.For_i(0, loop_n, 1))
        cpool = ctx.enter_context(tc.tile_pool(name="consts", bufs=1))
        dram = ctx.enter_context(tc.tile_pool(name="dram", bufs=1, space="DRAM"))
        xt_dram = [
            dram.tile([L, BC // 2], f32, name=f"xt_dram{h}") for h in range(2)
        ]

        cst = cpool.tile([128, CST_W], f32, name="cst")
        nc.sync.dma_start(cst[:], cst_in)
        idx_t = cpool.tile([128, NM * (GIDX // 16)], i16, name="idx_t")
        nc.sync.dma_start(idx_t[:], idx_in)

        ident = cst[:, CST_ID : CST_ID + 128]
        cnt_t = cst[:, CST_CNT : CST_CNT + 128]
        w1_t = cst[:, CST_W1 : CST_W1 + 32]
        b2_t = cst[:, CST_B2 : CST_B2 + 2]
        w2_t = cst[0:16, CST_W2 : CST_W2 + 256]
        b1_t = cst[0:16, CST_B1 : CST_B1 + 1]

        s_sb = cpool.tile([4, BC], f32, name="s_sb")
        wt_sb = cpool.tile([4, BC], f32, name="wt_sb")
        wb = [cpool.tile([128, BC], f32, name=f"wb{r}") for r in range(R)]

        # ---------------- phase 1: transpose x -> xt_dram, s = x @ cnt ----
        with (
            tc.tile_pool(name="xp", bufs=1) as xpool,
            tc.tile_pool(name="xtp", bufs=4) as xtpool,
            tc.tile_pool(name="pp1", bufs=6, space="PSUM") as pp1,
            tc.tile_pool(name="pps", bufs=1, space="PSUM") as pps,
        ):
            xs = []
            for j in range(4):
                xj = xpool.tile([128, L], f32, name=f"xs{j}")
                for h in range(4):
                    nc.sync.dma_start(
                        xj[:, h * 1024 : (h + 1) * 1024],
                        x_in[j * 128 : (j + 1) * 128, h * 1024 : (h + 1) * 1024],
                    )
                xs.append(xj)
            psum_s = pps.tile([4, BC], f32, name="psum_s")
            for half in range(2):
                for tq in range(NT // 4):
                    xt_t = xtpool.tile([128, 4, BC // 2], f32, name="xt_t")
                    for a in range(4):
                        t = 4 * tq + a
                        pt = pp1.tile([128, BC // 2], f32, name="pt")
                        for jj in range(2):
                            j = 2 * half + jj
                            nc.tensor.transpose(
                                pt[:, jj * 128 : (jj + 1) * 128],
                                xs[j][:, t * 128 : (t + 1) * 128],
                                ident,
                            )
                        nc.vector.tensor_copy(xt_t[:, a, :], pt[:])
                        nc.tensor.matmul(
                            psum_s[:, half * 256 : (half + 1) * 256],
                            cnt_t[:, 4 * t : 4 * t + 4],
                            xt_t[:, a, :],
                            start=(t == 0),
                            stop=(t == NT - 1),
                        )
                    for a in range(4):
                        t = 4 * tq + a
                        nc.scalar.dma_start(
                            xt_dram[half][t * 128 : (t + 1) * 128, :], xt_t[:, a, :]
                        )
            nc.vector.tensor_copy(s_sb[:], psum_s[:])

        # ---------------- SE MLP + softmax over rotations ------------------
        with (
            tc.tile_pool(name="mlp", bufs=1) as mpool,
            tc.tile_pool(name="ppm", bufs=1, space="PSUM") as ppm,
        ):
            sT = []
            for j in range(4):
                p_sT = ppm.tile([128, 4], f32, name="p_sT")
                nc.tensor.transpose(
                    p_sT[:], s_sb[:, j * 128 : (j + 1) * 128], cst[0:4, CST_ID : CST_ID + 4]
                )
                sTj = mpool.tile([128, 4], f32, name=f"sT{j}")
                nc.vector.tensor_copy(sTj[:], p_sT[:])
                sT.append(sTj)
            hs = []
            for b in range(BPC):
                p_h = ppm.tile([16, 4], f32, name="p_h")
                for hi in range(2):
                    nc.tensor.matmul(
                        p_h[:],
                        w1_t[:, hi * 16 : (hi + 1) * 16],
                        sT[2 * b + hi][:],
                        start=(hi == 0),
                        stop=(hi == 1),
                    )
                h_sb = mpool.tile([16, 4], f32, name=f"h{b}")
                nc.scalar.activation(h_sb[:], p_h[:], AF.Relu, bias=b1_t)
                hs.append(h_sb)
            p_sc = ppm.tile([128, 16], f32, name="p_sc")
            for b in range(BPC):
                for hi in range(2):
                    j = 2 * b + hi
                    nc.tensor.matmul(
                        p_sc[:, 4 * j : 4 * j + 4],
                        w2_t[:, hi * 128 : (hi + 1) * 128],
                        hs[b][:],
                        start=True, stop=True,
                    )
            sc_all = mpool.tile([128, 4, 4], f32, name="sc_all")
            b2ap = b2_t
            b2v = bass.AP(
                b2ap.tensor, b2ap.offset, [b2ap.ap[0], [0, 2], b2ap.ap[1], [0, 4]]
            )
            nc.vector.tensor_tensor(
                sc_all[:].rearrange("p (b hi) r -> p b hi r", b=2),
                p_sc[:].rearrange("p (b hi r) -> p b hi r", b=2, hi=2),
                b2v,
                op=ALU.add,
            )
            negmx = mpool.tile([128, 4], f32, name="negmx")
            nc.vector.reduce_max(negmx[:], sc_all[:], axis=AX.X, negate=True)
            nm = negmx[:]
            nmv = bass.AP(nm.tensor, nm.offset, [nm.ap[0], nm.ap[1], [0, 4]])
            nc.vector.tensor_tensor(sc_all[:], sc_all[:], nmv, op=ALU.add)
            e_all = mpool.tile([128, 4, 4], f32, name="e_all")
            nc.scalar.activation(
                e_all[:].rearrange("p a r -> p (a r)"),
                sc_all[:].rearrange("p a r -> p (a r)"),
                AF.Exp,
            )
            sm = mpool.tile([128, 4], f32, name="sm")
            nc.vector.reduce_sum(sm[:], e_all[:], axis=AX.X)
            rcp = mpool.tile([128, 4], f32, name="rcp")
            nc.vector.reciprocal(rcp[:], sm[:])
            rc = rcp[:]
            rcv = bass.AP(rc.tensor, rc.offset, [rc.ap[0], rc.ap[1], [0, 4]])
            W_all = mpool.tile([128, 4, 4], f32, name="W_all")
            nc.vector.tensor_tensor(W_all[:], e_all[:], rcv, op=ALU.mult)
            for j in range(4):
                p_wt = ppm.tile([4, 128], f32, name="p_wt")
                nc.tensor.transpose(p_wt[:], W_all[:, j, :], ident)
                nc.vector.tensor_copy(wt_sb[:, j * 128 : (j + 1) * 128], p_wt[:])
            for r in range(R):
                p_wb = ppm.tile([128, BC], f32, name="p_wb")
                nc.tensor.matmul(
                    p_wb[:],
                    cst[0:4, CST_SEL + r * 128 : CST_SEL + (r + 1) * 128],
                    wt_sb[:],
                    start=True, stop=True,
                )
                nc.vector.tensor_copy(wb[r][:], p_wb[:])

        # ---------------- phase 2: gather, transpose back, scale+sum ------
        with (
            tc.tile_pool(name="gp", bufs=5) as gpool,
            tc.tile_pool(name="op", bufs=1) as opool,
            tc.tile_pool(name="pp2", bufs=6, space="PSUM") as pp2,
        ):
            outs = [opool.tile([128, L], f32, name=f"os{j}") for j in range(4)]
            NGI = M_CHUNK // 128
            HB = BC // 2
            for m in range(NM):
                gh = []
                for half in range(2):
                    g = gpool.tile([128, NGI * R, HB], f32, name=f"g{half}")
                    nc.gpsimd.dma_gather(
                        g[:],
                        xt_dram[half][:],
                        idx_t[:, m * (GIDX // 16) : (m + 1) * (GIDX // 16)],
                        GIDX,
                        GIDX,
                        HB,
                    )
                    gh.append(g)
                for half in range(2):
                    g = gh[half]
                    for r in range(R):
                        gs = g[:, NGI * r : NGI * (r + 1), :]
                        wba = wb[r][:, half * HB : (half + 1) * HB]
                        wv = bass.AP(
                            wba.tensor, wba.offset, [wba.ap[0], [0, NGI], wba.ap[1]]
                        )
                        nc.vector.tensor_tensor(gs, gs, wv, op=ALU.mult)
                    for jj in range(2):
                        j = 2 * half + jj
                        po = pp2.tile([128, M_CHUNK], f32, name="po")
                        for gi in range(NGI):
                            for r in range(R):
                                nc.tensor.matmul(
                                    po[:, gi * 128 : (gi + 1) * 128],
                                    g[:, NGI * r + gi, jj * 128 : (jj + 1) * 128],
                                    ident,
                                    is_transpose=True,
                                    start=(r == 0),
                                    stop=(r == R - 1),
                                )
                        nc.scalar.copy(
                            outs[j][:, m * M_CHUNK : (m + 1) * M_CHUNK], po[:]
                        )
                if m % 4 == 3:
                    q = m // 4
                    W4 = 4 * M_CHUNK
                    for j in range(4):
                        nc.sync.dma_start(
                            out[j * 128 : (j + 1) * 128, q * W4 : (q + 1) * W4],
                            outs[j][:, q * W4 : (q + 1) * W4],
                        )

    nc.compile()
    return nc


def _host_prep(x, rot_idx, w1, b1, w2, b2):
    x = np.asarray(x, dtype=np.float32)
    rot_idx = np.asarray(rot_idx, dtype=np.int64)
    w1 = np.asarray(w1, dtype=np.float32)
    b1 = np.asarray(b1, dtype=np.float32)
    w2 = np.asarray(w2, dtype=np.float32)
    b2 = np.asarray(b2, dtype=np.float32)

    cnt = np.zeros((R, L), dtype=np.float32)
    for r in range(R):
        cnt[r] = np.bincount(rot_idx[r], minlength=L).astype(np.float32)
    cnt /= np.float32(L)
    # cnt_sb[p, 4t+r] = cnt[r, t*128+p]
    cnt_sb = np.ascontiguousarray(
        cnt.T.reshape(NT, 128, R).transpose(1, 0, 2).reshape(128, 128)
    )

    cst = np.zeros((128, CST_W), dtype=np.float32)
    cst[:, CST_ID : CST_ID + 128] = np.eye(128, dtype=np.float32)
    cst[:, CST_CNT : CST_CNT + 128] = cnt_sb
    cst[:, CST_W1 : CST_W1 + 32] = (
        w1.reshape(2, 128, RED).transpose(1, 0, 2).reshape(128, 2 * RED)
    )
    cst[:, CST_B2 : CST_B2 + 2] = b2.reshape(2, 128).T
    cst[0:16, CST_W2 : CST_W2 + 256] = w2
    cst[0:16, CST_B1] = b1
    for r in range(R):
        cst[r, CST_SEL + r * 128 : CST_SEL + (r + 1) * 128] = 1.0

    # gather index table: per chunk m, linear order [r0 l's..., r1 l's, ...],
    # wrapped idx_layout[p, s] = lin[s*16 + p], replicated over 8 core groups
    idx_sb = np.zeros((128, NM * (GIDX // 16)), dtype=np.int16)
    for m in range(NM):
        lin = np.concatenate(
            [rot_idx[r, m * M_CHUNK : (m + 1) * M_CHUNK] for r in range(R)]
        ).astype(np.int16)
        block = lin.reshape(GIDX // 16, 16).T  # [16, 64]
        idx_sb[:, m * (GIDX // 16) : (m + 1) * (GIDX // 16)] = np.tile(block, (8, 1))

    shared = {"cst": cst, "idx": idx_sb}
    in_maps = []
    for c in range(NCORES):
        m = dict(shared)
        m["x"] = np.ascontiguousarray(x[c * BPC : (c + 1) * BPC].reshape(BC, L))
        in_maps.append(m)
    return in_maps


def kernel(x, rot_idx, w1, b1, w2, b2, _trace=False):
    from concourse import bass_utils

    in_maps = _host_prep(x, rot_idx, w1, b1, w2, b2)
    if "nc" not in _NC_CACHE:
        _NC_CACHE["nc"] = _build_nc()
    nc = _NC_CACHE["nc"]
    res = bass_utils.run_bass_kernel_spmd(
        nc, in_maps, core_ids=list(range(NCORES)), trace=_trace
    )
    out = np.empty((B, C, L), dtype=np.float32)
    for c in range(NCORES):
        out[c * BPC : (c + 1) * BPC] = res.results[c]["out"].reshape(BPC, C, L)
    if _trace:
        kernel.last_results = res
    return out



# revision 7
# speedup vs baseline: 2.5967x; 2.5967x over previous
"""Trainium2 Bass kernel for CyclicShiftConv (Hilbert-rotation SE attention).

out[b,c,l] = sum_r softmax_r(MLP(mean_l x[b,c,rot_idx[r,l]]))[b,c,r] * x[b,c,rot_idx[r,l]]

Key identities exploited:
  1. Every row of rot_idx is a permutation of [0, L), so
     mean_l x[b,c,rot_idx[r,l]] == mean_l x[b,c,l] for every r: the SE-MLP
     sees identical inputs for all rotations and the softmax weights are
     exactly 1/R = 0.25. The whole MLP/softmax stage folds into a constant.
  2. The Hilbert-rotation permutations are 128-block-sparse: within any
     128-column destination tile all source indices fall in at most two
     128-row source tiles, in a handful of aligned runs (~128 runs total
     for the three non-identity rotations). Each run is realized as one
     PE matmul out[bc, d0:d0+len] = x^T_tile.T @ B_run where B_run is a
     0/1 column-selector block, so the gather costs ~1 PE cycle/column in
     bf16 and lands directly in natural [bc, l] layout.

Per-core schedule (8 cores, data-parallel over batch; 512 (b,c) rows/core):
  - DMA x in 16 [128,1024] tiles; PE-transpose into 32 bf16 tiles
    T_s = 0.25 * x[:, s*128:(s+1)*128]^T (scale folded into the PSUM
    eviction on the scalar engine).
  - Selector matrix B (one 1 per column at the in-tile source offset) is
    built on-device: a tiny bf16 index row is PE-broadcast to 128
    partitions via a rank-1 matmul, then compared against an iota column
    with tensor_scalar(is_equal) on DVE/GpSimd.
  - Per 512-column chunk and 128-row bc block: the three rotations'
    run-matmuls accumulate into one PSUM bank (per-element: rotation 1
    writes every column exactly once with start=True, rotations 2/3
    accumulate), then one fused eviction out = 0.25*x + psum
    (scalar_tensor_tensor) feeds the output DMA.

HBM traffic is 16 MiB/core (8 in + 8 out), the roofline for this op.
"""

import sys

for _p in ("/opt/trn_rl_repo", "/opt/pypackages"):
    if _p not in sys.path:
        sys.path.append(_p)

import numpy as np

B, C, L, R = 16, 256, 4096, 4
NCORES = 8
BPC = B // NCORES          # samples per core
BC = BPC * C               # 512 rows per core
NJ = BC // 128             # 4 bc blocks
NS = L // 128              # 32 source tiles
CHUNK = 512
NCH = L // CHUNK           # 8 output chunks
XW = 1024                  # x DMA tile width
NCB = L // XW              # 4 column blocks

_CACHE = {}


def _plan(rot_idx):
    """Derive the run decomposition of the non-identity permutation rows."""
    rot = np.asarray(rot_idx, dtype=np.int64)
    assert rot.shape == (R, L)
    ar = np.arange(L)
    for r in range(R):
        assert len(np.unique(rot[r])) == L, "rot_idx rows must be permutations"
    id_rows = [r for r in range(R) if np.array_equal(rot[r], ar)]
    perm_rows = [r for r in range(R) if r not in id_rows]
    runs_by_chunk = [[] for _ in range(NCH)]  # (ri, d0, len, s, col0)
    cols = []
    ncol = 0
    for ri, r in enumerate(perm_rows):
        st = rot[r] // 128
        brk = np.nonzero((np.diff(st) != 0) | ((ar[1:] % CHUNK) == 0))[0] + 1
        bounds = np.concatenate([[0], brk, [L]])
        for a, b in zip(bounds[:-1], bounds[1:]):
            a, b = int(a), int(b)
            s = int(st[a])
            c = a // CHUNK
            runs_by_chunk[c].append((ri, a - c * CHUNK, b - a, s, ncol))
            cols.append(rot[r, a:b] - 128 * s)
            ncol += b - a
    cols = np.concatenate(cols).astype(np.float32) if cols else np.zeros(0, np.float32)
    assert ncol == len(perm_rows) * L
    return id_rows, len(perm_rows), runs_by_chunk, cols, ncol


def _build_nc(nperm, runs_by_chunk, ncol, n_id):
    import concourse.bass as bass
    import concourse.mybir as mybir
    from concourse import bacc
    from concourse.tile import TileContext
    from contextlib import ExitStack

    f32 = mybir.dt.float32
    bf16 = mybir.dt.bfloat16
    AF = mybir.ActivationFunctionType
    ALU = mybir.AluOpType

    nc = bacc.Bacc(
        "TRN2",
        target_bir_lowering=False,
        debug=False,
        enable_asserts=False,
        num_devices=NCORES,
    )

    x_in = nc.dram_tensor("x", [BC, L], f32, kind="ExternalInput").ap()
    cstb_in = nc.dram_tensor("cstb", [1, ncol + 128], bf16, kind="ExternalInput").ap()
    cst32_in = nc.dram_tensor("cst32", [128, 129], f32, kind="ExternalInput").ap()
    out = nc.dram_tensor("out", [BC, L], f32, kind="ExternalOutput").ap()

    with TileContext(nc) as tc, ExitStack() as ctx:
        cpool = ctx.enter_context(tc.tile_pool(name="consts", bufs=1))
        xpool = ctx.enter_context(tc.tile_pool(name="xp", bufs=1))
        tpool = ctx.enter_context(tc.tile_pool(name="tp", bufs=1))
        bpool = ctx.enter_context(tc.tile_pool(name="bp", bufs=1))
        opool = ctx.enter_context(tc.tile_pool(name="op", bufs=6))

        cstb = cpool.tile([1, ncol + 128], bf16, name="cstb")
        nc.sync.dma_start(cstb[:], cstb_in)
        cst32 = cpool.tile([128, 129], f32, name="cst32")
        nc.sync.dma_start(cst32[:], cst32_in)
        iota = cst32[:, 0:1]
        ident = cst32[:, 1:129]
        ones_row = cstb[0:1, ncol : ncol + 128]

        # ---- build the 0/1 selector matrix B on-device --------------------
        B_all = bpool.tile([128, max(ncol, 512)], bf16, name="B_all")
        NB = ncol // 512
        with tc.tile_pool(name="ppB", bufs=2, space="PSUM") as ppB:
            for k in range(NB):
                pb = ppB.tile([128, 512], f32, name="pb")
                nc.tensor.matmul(
                    pb[:],
                    ones_row,
                    cstb[0:1, k * 512 : (k + 1) * 512],
                    start=True,
                    stop=True,
                )
                # GPSIMD cannot read PSUM; these stay on DVE.
                nc.vector.tensor_scalar(
                    B_all[:, k * 512 : (k + 1) * 512], pb[:], iota, None,
                    op0=ALU.is_equal,
                )

        # ---- x in, transposes, permutation matmuls, fused eviction --------
        with (
            tc.tile_pool(name="ppT", bufs=2, space="PSUM") as ppT,
            tc.tile_pool(name="ppP", bufs=4, space="PSUM") as ppP,
        ):
            xs = [[None] * NCB for _ in range(NJ)]
            for cb in range(NCB):
                for j in range(NJ):
                    xt = xpool.tile([128, XW], f32, name=f"x{j}_{cb}")
                    # SWDGE path: keeps the SP sequencer free for output DMAs
                    # (HWDGE issue costs ~1.3us of SP.SEQ per DMA).
                    nc.gpsimd.dma_start(
                        xt[:], x_in[j * 128 : (j + 1) * 128, cb * XW : (cb + 1) * XW]
                    )
                    xs[j][cb] = xt

            Ts = []
            for cb in range(NCB):
                for si in range(XW // 128):
                    s = cb * (XW // 128) + si
                    pT = ppT.tile([128, 512], f32, name="pT")
                    for j in range(NJ):
                        nc.tensor.transpose(
                            pT[:, j * 128 : (j + 1) * 128],
                            xs[j][cb][:, si * 128 : (si + 1) * 128],
                            ident,
                        )
                    T_s = tpool.tile([128, 512], bf16, name=f"T{s}")
                    nc.scalar.activation(T_s[:], pT[:], AF.Copy, bias=0.0, scale=0.25)
                    Ts.append(T_s)

            for c in range(NCH):
                rl = sorted(runs_by_chunk[c])
                ob = opool.tile([128, NJ, CHUNK], f32, name="ob")
                for j in range(NJ):
                    pp = ppP.tile([128, CHUNK], f32, name="pp")
                    # One accumulation group per bank: start only on the first
                    # matmul (pending-zeroes the whole 2KB zero region), stop
                    # only on the last; partial-coverage writes accumulate via
                    # per-element has_written.
                    for qi, (ri, d0, ln, s, col0) in enumerate(rl):
                        nc.tensor.matmul(
                            pp[:, d0 : d0 + ln],
                            Ts[s][:, j * 128 : (j + 1) * 128],
                            B_all[:, col0 : col0 + ln],
                            start=(qi == 0),
                            stop=(qi == len(rl) - 1),
                        )
                    xsrc = xs[j][c // 2][:, (c % 2) * CHUNK : (c % 2 + 1) * CHUNK]
                    if n_id:
                        nc.vector.scalar_tensor_tensor(
                            ob[:, j, :], xsrc, 0.25 * n_id, pp[:],
                            op0=ALU.mult, op1=ALU.add,
                        )
                    else:
                        nc.vector.tensor_copy(ob[:, j, :], pp[:])
                # One DMA per chunk: SBUF [128, NJ, 512] -> DRAM rows
                # j*128+p, cols c*512+d (3-D access pattern, 2KiB chunks).
                oap = bass.AP(
                    out.tensor,
                    out.offset + c * CHUNK,
                    [[L, 128], [128 * L, NJ], [1, CHUNK]],
                )
                nc.sync.dma_start(oap, ob[:])

    nc.compile()
    return nc


def _host_prep(x, rot_idx):
    x = np.asarray(x, dtype=np.float32)
    id_rows, nperm, runs_by_chunk, cols, ncol = _plan(rot_idx)

    # bf16 row: per-column in-tile source offsets (< 128, exactly
    # representable) followed by 128 ones (broadcast stationary).
    import ml_dtypes

    cstb = np.zeros((1, ncol + 128), dtype=ml_dtypes.bfloat16)
    cstb[0, :ncol] = cols.astype(ml_dtypes.bfloat16)
    cstb[0, ncol:] = np.float32(1.0).astype(ml_dtypes.bfloat16)

    cst32 = np.zeros((128, 129), dtype=np.float32)
    cst32[:, 0] = np.arange(128, dtype=np.float32)
    cst32[:, 1:129] = np.eye(128, dtype=np.float32)

    shared = {"cstb": cstb, "cst32": cst32}
    in_maps = []
    for c in range(NCORES):
        m = dict(shared)
        m["x"] = np.ascontiguousarray(x[c * BPC : (c + 1) * BPC].reshape(BC, L))
        in_maps.append(m)
    return in_maps, (nperm, runs_by_chunk, ncol, len(id_rows))


def kernel(x, rot_idx, w1, b1, w2, b2, _trace=False):
    from concourse import bass_utils

    in_maps, plan = _host_prep(x, rot_idx)
    key = hash(np.asarray(rot_idx).tobytes())
    if key not in _CACHE:
        nperm, runs_by_chunk, ncol, n_id = plan
        _CACHE[key] = _build_nc(nperm, runs_by_chunk, ncol, n_id)
        _CACHE["nc"] = _CACHE[key]
    nc = _CACHE[key]
    res = bass_utils.run_bass_kernel_spmd(
        nc, in_maps, core_ids=list(range(NCORES)), trace=_trace
    )
    out = np.empty((B, C, L), dtype=np.float32)
    for c in range(NCORES):
        out[c * BPC : (c + 1) * BPC] = res.results[c]["out"].reshape(BPC, C, L)
    if _trace:
        kernel.last_results = res
    return out
